# revision 66
# baseline (speedup 1.0000x reference)
"""GNN scatter-mean + Linear kernel for Trainium2, 8 NeuronCores.

Strategy (node-sharded, fp8 DoubleRow, no collectives):
  - CPU: sort edges by destination node, bucket per core (each core owns 1250
    contiguous nodes = 10 tiles of 128). Edge features are shipped RAW
    (unscaled) quantized to fp8 e4m3 with error-feedback (noise-shaped)
    rounding along each node's edge chain: the running quantization error is
    carried into the next edge of the same node, so the device-side segment
    sum sees only ~one ulp of error instead of sqrt(count) ulps. The 1/count
    mean division (and the fp8 range scale) is applied on-device per node
    after aggregation.
  - Slots hold 256 edges as [128 partitions, 2 packed] consumed by a single
    DoubleRow fp8 matmul (2 contractions of K=128 per instruction, 128 PE
    cycles per slot): identity slots (partition p, both halves -> node p) use
    one constant [128,2,128] fp8 weight tile loaded once (redundant Ldweights
    stripped post-compile); overflow edges use per-slot one-hot weights built
    on DVE (iota + is_equal, one op per packed half). PSUM accumulates fp32.
  - The whole edge stream (~10.7 MB/core) fits in SBUF, so every chunk DMA
    is triggered upfront with no pool recycling and the PE just follows the
    stream front. Consts are packed into 3 DMAs and the output into 2 that
    are emitted last, keeping the total DMA count low: past ~2 outstanding
    per DMAHW lane the tile scheduler injects cap-gate waits into engine
    streams that couple early compute to late chunk completions.
  - Per node tile (software-pipelined one tile behind the aggregation so the
    in-order PE queue never stalls on the finalize chain): evict PSUM on the
    ACT engine via activation-with-scale using the per-node 1/(count*SCALE)
    vector (fp16 out), transpose via PE, apply the 256x256 Linear (2 K-chunk
    fp16 matmuls; bias via a K=1 ones-row matmul only when nonzero), copy to
    the staging buffer on ACT, batched DMA out.
  - CPU: concatenate the 8 per-core [1250, 256] blocks.

At this point the kernel is bound by chip-aggregate HBM bandwidth (8 cores
x 10.7 MB streaming concurrently) plus ~13 us of fixed NEFF init/teardown;
the slowest core is set by DMA arbitration luck (measured ~60 us max,
vs 92.9 us for the fp16 identity-slot baseline).
"""

import sys

sys.path.insert(0, "/opt/trn_rl_repo")

from contextlib import ExitStack

import ml_dtypes
import numpy as np

N_NODES = 10000
N_EDGES = 320000
FEAT = 256
NCORES = 8
P = 128
NPC = (N_NODES + NCORES - 1) // NCORES  # 1250 nodes per core
NTILES = (NPC + P - 1) // P  # 10 node tiles per core
CH = 16  # slots per steady-state DMA chunk (16 * 128 * 2 * 256 * 1B = 1 MiB)
SCALE = 32.0  # fp8 range scale; folded into the on-device recip multiply
W_OVF = 1.05  # cost weight of an overflow slot vs an identity slot

FP8 = ml_dtypes.float8_e4m3fn


def _plan(dst):
    """Choose the shared program structure from the destination indices.

    Slots carry 256 edges ([128 partitions, 2 packed]). caps[t] identity
    slots cover up to 2*caps[t] edges per node; k_ovf[t] one-hot slots take
    the spill. Shared across all 8 cores so one SPMD program serves every
    core.
    """
    perm = np.argsort(dst, kind="stable")
    dst_sorted = dst[perm]
    counts = np.bincount(dst, minlength=N_NODES)

    tile_ranges = []
    for c in range(NCORES):
        rng = []
        for t in range(NTILES):
            n0 = c * NPC + t * P
            n1 = min(c * NPC + min((t + 1) * P, NPC), N_NODES)
            lo = int(np.searchsorted(dst_sorted, n0, side="left"))
            hi = int(np.searchsorted(dst_sorted, n1, side="left"))
            rng.append((lo, hi, n0, n1))
        tile_ranges.append(rng)

    caps, k_ovf = [], []
    for t in range(NTILES):
        cnts = [counts[rng[t][2] : rng[t][3]] for rng in tile_ranges]
        best = None
        for C in range(1, 129):
            ovf_slots = max(
                -(-int(np.maximum(cc - 2 * C, 0).sum()) // (2 * P)) if cc.size else 0
                for cc in cnts
            )
            cost = C + W_OVF * ovf_slots
            if best is None or cost < best[0]:
                best = (cost, C, ovf_slots)
        _, C, ovf_slots = best
        caps.append(C)
        k_ovf.append(ovf_slots)

    base = [0] * (NTILES + 1)
    cur = 0
    for t in range(NTILES):
        base[t] = cur
        cur += caps[t] + k_ovf[t]
    base[NTILES] = cur
    return perm, dst_sorted, counts, tile_ranges, caps, k_ovf, base, cur


def _chunk_schedule(nslot):
    """All chunks are triggered upfront. Progressive sizes keep the PE fed
    continuously from ~1.5us in (instead of waiting for a first big
    transfer), a small tail chunk keeps the last tile's wait short. Keep the
    chunk count modest: the tile scheduler round-robins every DMA over 8
    DMAHW lanes and inserts waits on the 2nd+ same-lane occupant, so deep
    chunk lists serialize on chunk completions (benign) but anything
    compute-gated (the out-writes) must come LAST in emission order."""
    head = [2, 2, 4, 8, 16]
    tail = [6, 4]
    sizes = []
    rem = nslot - sum(tail)
    for sz in head:
        if rem <= 0:
            break
        take = min(sz, rem)
        sizes.append(take)
        rem -= take
    n_steady = max(1, -(-rem // CH))
    for i in range(n_steady):
        take = rem // (n_steady - i)
        sizes.append(take)
        rem -= take
    for sz in tail:
        sizes.append(sz)
    assert sum(sizes) == nslot, sizes
    return sizes


def _slot_to_chunk(chunk_sizes):
    m = []
    for ci, sz in enumerate(chunk_sizes):
        for cl in range(sz):
            m.append((ci, cl))
    return m


def _quantize_ef(src, perm, dst_sorted, counts):
    """fp8 e4m3 quantization with per-(node,feature) error feedback.

    Edges are processed in sorted order; the rounding error of edge r of a
    node is added to edge r+1 before rounding, telescoping the segment-sum
    error down to the final edge's single rounding error. Vectorized across
    nodes by rank. Returns codes aligned with the SORTED edge order, plus a
    trailing all-zeros pad row (gather index N_EDGES)."""
    x = src[perm].astype(np.float32) * SCALE
    starts = np.searchsorted(dst_sorted, np.arange(N_NODES)).astype(np.int64)
    q = np.empty((N_EDGES + 1, FEAT), dtype=FP8)
    q[N_EDGES] = 0.0
    carry = np.zeros((N_NODES, FEAT), dtype=np.float32)
    maxc = int(counts.max())
    nodes_all = np.arange(N_NODES)
    for r in range(maxc):
        sel = nodes_all[counts > r]
        eidx = starts[sel] + r
        t = x[eidx] + carry[sel]
        np.clip(t, -239.0, 239.0, out=t)
        qv = t.astype(FP8)
        q[eidx] = qv
        carry[sel] = t - qv.astype(np.float32)
    return q


def _build_program(caps, k_ovf, base, chunk_sizes, nslot, has_bias, dedup=True):
    from concourse import bacc, mybir
    import concourse.tile as tile

    f32 = mybir.dt.float32
    f16 = mybir.dt.float16
    f8 = mybir.dt.float8e4
    eq = mybir.AluOpType.is_equal
    Ident = mybir.ActivationFunctionType.Identity
    DR = mybir.MatmulPerfMode.DoubleRow

    nc = bacc.Bacc("TRN2", target_bir_lowering=False, debug=False)

    src_drams = [
        nc.dram_tensor(f"src{i}", [P, ch, 2, FEAT], f8, kind="ExternalInput")
        for i, ch in enumerate(chunk_sizes)
    ]
    # consts are packed into 3 DMAs (f16 / f32 / fp8) to keep the total DMA
    # count low (DMAHW lane pressure, see _chunk_schedule).
    # cf16 segments of P: [iota | identt | wt(4) | bias(2) | ones-row | pad]
    cf16_d = nc.dram_tensor("cf16", [P, 10, P], f16, kind="ExternalInput")
    cf32_d = nc.dram_tensor("cf32", [P, 2 * nslot + NTILES], f32, kind="ExternalInput")
    identw_d = nc.dram_tensor("identw", [P, 2, P], f8, kind="ExternalInput")
    out_d = nc.dram_tensor("out", [P, NTILES, 2, P], f16, kind="ExternalOutput")

    with tile.TileContext(nc) as tc, ExitStack() as ctx:
        const = ctx.enter_context(tc.tile_pool(name="const", bufs=1))

        srcp = ctx.enter_context(
            tc.tile_pool(name="srcp", bufs=min(6, len(chunk_sizes)))
        )
        ohp = ctx.enter_context(tc.tile_pool(name="ohp", bufs=12))
        meanp = ctx.enter_context(tc.tile_pool(name="meanp", bufs=2))
        mtp = ctx.enter_context(tc.tile_pool(name="mtp", bufs=2))
        outp = ctx.enter_context(tc.tile_pool(name="outp", bufs=1))
        ps_agg = ctx.enter_context(tc.tile_pool(name="ps_agg", bufs=2, space="PSUM"))
        ps_t = ctx.enter_context(tc.tile_pool(name="ps_t", bufs=2, space="PSUM"))
        ps_out = ctx.enter_context(tc.tile_pool(name="ps_out", bufs=2, space="PSUM"))

        identw_sb = const.tile([P, 2, P], f8)
        nc.scalar.dma_start(identw_sb[:], identw_d[:])
        cf16 = const.tile([P, 10, P], f16)
        nc.scalar.dma_start(cf16[:], cf16_d[:])
        cf32 = const.tile([P, 2 * nslot + NTILES], f32)
        nc.scalar.dma_start(cf32[:], cf32_d[:])


        # the whole edge stream lives in SBUF: trigger every chunk upfront so
        # the DMA engines stream back-to-back with no flow-control coupling
        chunk_tiles = []
        for i, ch in enumerate(chunk_sizes):
            ct = srcp.tile([P, ch, 2, FEAT], f8, tag="src_chunk")
            nc.sync.dma_start(ct[:], src_drams[i][:])
            chunk_tiles.append(ct)

        s2c = _slot_to_chunk(chunk_sizes)

        # single output staging buffer; batched out-DMAs are emitted AFTER all
        # chunk triggers in scheduling order so the round-robin DMAHW lane
        # sems never make a chunk trigger wait on a compute-gated out-write
        ob_all = outp.tile([P, NTILES, 2, P], f16)
        OUT_SPLIT = 9

        def finalize(t, agg):
            # tile-finalize chain: PSUM-evict with the per-node 1/(count*S)
            # scale + the Linear. Evict/copy ops run on ACT (scalar) so the
            # DVE keeps its throughput for one-hot builds; bias is injected
            # as a K=1 matmul (ones-row x bias-row) that also initializes
            # the PSUM.
            mean = meanp.tile([P, FEAT], f16)
            rc = 2 * nslot + t
            nc.scalar.activation(mean[:], agg[:], Ident, scale=cf32[:, rc : rc + 1])
            tp = ps_t.tile([P, 2, P], f16)
            nc.tensor.transpose(tp[:, 0, :], mean[:, 0:P], cf16[:, 1, :])
            nc.tensor.transpose(tp[:, 1, :], mean[:, P : 2 * P], cf16[:, 1, :])
            mt = mtp.tile([P, 2, P], f16)
            nc.scalar.copy(mt[:], tp[:])
            op_ = ps_out.tile([P, 2, P], f32)
            if has_bias:
                nc.tensor.matmul(
                    op_[:], cf16[0:1, 8, :], cf16[0:1, 6:8, :], start=True, stop=False
                )
            nc.tensor.matmul(
                op_[:], mt[:, 0, :], cf16[:, 2:4, :], start=not has_bias, stop=False
            )
            nc.tensor.matmul(op_[:], mt[:, 1, :], cf16[:, 4:6, :], start=False, stop=True)
            nc.scalar.copy(ob_all[:, t], op_[:])
            if t == OUT_SPLIT - 1:
                nc.scalar.dma_start(out_d[:, 0:OUT_SPLIT], ob_all[:, 0:OUT_SPLIT])
            elif t == NTILES - 1:
                nc.scalar.dma_start(
                    out_d[:, OUT_SPLIT:NTILES], ob_all[:, OUT_SPLIT:NTILES]
                )

        # software pipelining: tile t's finalize is emitted AFTER tile t+2's
        # aggregation matmuls, so the in-order PE queue never stalls waiting
        # for the ACT engine's eviction/copy of a previous tile
        from collections import deque

        pending = deque()
        for t in range(NTILES):
            agg = ps_agg.tile([P, FEAT], f32)
            kst = caps[t] + k_ovf[t]
            for k in range(kst):
                s = base[t] + k
                ci, cl = s2c[s]
                ct = chunk_tiles[ci]
                if k < caps[t]:
                    lhsT = identw_sb[:]
                else:
                    oh = ohp.tile([P, 2, P], f8)
                    nc.vector.tensor_scalar(
                        oh[:, 0, :], cf16[:, 0, :], cf32[:, 2 * s : 2 * s + 1], None, eq
                    )
                    nc.vector.tensor_scalar(
                        oh[:, 1, :], cf16[:, 0, :], cf32[:, 2 * s + 1 : 2 * s + 2], None, eq
                    )
                    lhsT = oh[:]
                nc.tensor.matmul(
                    agg[:],
                    lhsT,
                    ct[:, cl],
                    start=(k == 0),
                    stop=(k == kst - 1),
                    perf_mode=DR,
                )
            pending.append((t, agg))
            if len(pending) > 1:
                finalize(*pending.popleft())
        while pending:
            finalize(*pending.popleft())

    nc.compile()
    if dedup:
        _postprocess_module(nc)
    return nc


def _postprocess_module(nc):
    """Two post-compile rewrites of the module JSON:

    1. Remove back-to-back redundant Ldweights on the PE stream (same weights
       AP, no new semaphore obligations): identity-slot chains reload the
       same stationary operand; Matmult keeps the last loaded weights.
    2. Hoist the leading wait-free DMA triggers (first src chunks + consts)
       out of the Tile body into `main` ahead of the all-engine init barrier
       so the first bytes stream during engine init."""
    import orjson
    from concourse import mybir

    raw = nc.to_json()
    removed = 0
    for fn in raw["functions"]:
        for blk in fn["blocks"]:
            out = []
            last_sig = None
            enforced = {}  # sem id -> max wait value already enforced on PE
            for inst in blk["instructions"]:
                if inst.get("engine") == "PE":
                    sync = inst.get("sync_info") or {}
                    waits = sync.get("on_wait") or []
                    if inst.get("opcode") == "Ldweights":
                        ups = sync.get("on_update") or []
                        sig = orjson.dumps(
                            {
                                k: v
                                for k, v in inst.items()
                                if k not in ("name", "debug", "sync_info")
                            },
                            option=orjson.OPT_SORT_KEYS,
                        )
                        if (
                            sig == last_sig
                            and not ups
                            and all(
                                w.get("sync_type") == "semaphore"
                                and isinstance(w.get("wait_value"), int)
                                and enforced.get(w["id"], -1) >= w["wait_value"]
                                for w in waits
                            )
                        ):
                            removed += 1
                            continue
                        last_sig = sig
                    for w in waits:
                        if w.get("sync_type") == "semaphore" and isinstance(
                            w.get("wait_value"), int
                        ):
                            enforced[w["id"]] = max(
                                enforced.get(w["id"], -1), w["wait_value"]
                            )
                out.append(inst)
            blk["instructions"] = out

    for fn in raw["functions"]:
        blocks = {b["name"]: b for b in fn["blocks"]}
        main = blocks.get("main")
        body = None
        for b in fn["blocks"]:
            if b["name"] != "main" and len(b["instructions"]) > 100:
                body = b
        if main is None or body is None:
            continue
        hoist = []
        kept = []
        for idx, inst in enumerate(body["instructions"]):
            if len(hoist) >= 10 or idx > 60:
                kept.extend(body["instructions"][idx:])
                break
            sync = inst.get("sync_info") or {}
            if inst.get("opcode") == "DMACopy" and not (sync.get("on_wait") or []):
                hoist.append(inst)
            else:
                kept.append(inst)
        if not hoist:
            continue
        body["instructions"] = kept
        mi = main["instructions"]
        pos = next(
            (i for i, x in enumerate(mi) if x.get("opcode") == "Drain"), len(mi)
        )
        main["instructions"] = mi[:pos] + hoist + mi[pos:]

    nc.m = mybir.parse_bytes(orjson.dumps(raw))
    return removed


def _prepare(inputs, dedup=True):
    """CPU-side sharding: returns (nc, in_maps) ready for SPMD dispatch."""
    src = np.asarray(inputs["source_node_representation_with_coefficient"])
    edge_index = np.asarray(inputs["edge_index"])
    W = np.asarray(inputs["W"], dtype=np.float32)
    b = np.asarray(inputs["b"], dtype=np.float32)
    assert src.shape == (N_EDGES, FEAT) and edge_index.shape == (2, N_EDGES)

    dst = edge_index[1].astype(np.int64)
    perm, dst_sorted, counts, tile_ranges, caps, k_ovf, base, nslot = _plan(dst)

    q = _quantize_ef(src, perm, dst_sorted, counts)  # [E+1, F] fp8, sorted order

    chunk_sizes = _chunk_schedule(nslot)
    has_bias = bool(np.any(b != 0))
    nc = _build_program(
        caps, k_ovf, base, chunk_sizes, nslot, has_bias, dedup=dedup
    )

    # cf16 = [iota | identt | wt(4) | bias(2) | ones-row | pad], segments [P, P]
    cf16_tile = np.zeros((P, 10, P), dtype=np.float16)
    cf16_tile[:, 0, :] = np.arange(P, dtype=np.float16)[None, :]
    cf16_tile[:, 1, :] = np.eye(P, dtype=np.float16)
    cf16_tile[:, 2:6, :] = (
        W.T.reshape(2, P, 2, P).transpose(1, 0, 2, 3).reshape(P, 4, P)
    ).astype(np.float16)
    cf16_tile[:, 6:8, :] = np.broadcast_to(b, (P, FEAT)).reshape(P, 2, P).astype(
        np.float16
    )
    cf16_tile[0, 8, :] = 1.0
    identw_tile = np.zeros((P, 2, P), dtype=FP8)
    for j in range(2):
        identw_tile[np.arange(P), j, np.arange(P)] = 1.0

    # recip[p, t] = 1 / (max(count,1) * SCALE) for node t*128+p of this core
    pad = N_EDGES  # index of the all-zeros pad row in q

    in_maps = []
    for c in range(NCORES):
        pos = np.full((nslot, P, 2), pad, dtype=np.int64)  # sorted-order edge idx
        rel = np.zeros((nslot, P, 2), dtype=np.int64)
        for t in range(NTILES):
            lo, hi, n0, n1 = tile_ranges[c][t]
            n = hi - lo
            rows = n1 - n0
            b0 = base[t]
            C = caps[t]
            if n == 0:
                continue
            d_rel = dst_sorted[lo:hi] - n0  # sorted, in [0, rows)
            starts = np.searchsorted(d_rel, np.arange(rows))
            cnt_p = np.diff(np.append(starts, n))
            # identity slots: slot k half j, partition p <- edge 2k+j of node p
            kk = (2 * np.arange(C)[:, None, None] + np.arange(2)[None, None, :])
            valid = kk < cnt_p[None, :, None]  # [C, rows, 2]
            idx = np.minimum(starts[None, :, None] + kk, n - 1)
            pos[b0 : b0 + C, :rows] = np.where(valid, lo + idx, pad)
            # overflow edges: rank >= 2C within their node, packed densely
            rank = np.arange(n) - starts[d_rel]
            om = rank >= 2 * C
            novf = int(om.sum())
            if novf:
                ob0 = b0 + C
                tend = b0 + C + k_ovf[t]
                flat_pos = pos[ob0:tend].reshape(-1)
                flat_rel = rel[ob0:tend].reshape(-1)
                flat_pos[:novf] = lo + np.nonzero(om)[0]
                flat_rel[:novf] = d_rel[om]

        srcg = q[pos.reshape(-1)]  # [(nslot*P*2), F] fp8

        node0 = c * NPC
        cnt_core = np.zeros(NTILES * P, dtype=np.float64)
        ncv = min(NPC, N_NODES - node0)
        cnt_core[:ncv] = counts[node0 : node0 + ncv]
        recip = (1.0 / (np.maximum(cnt_core, 1.0) * SCALE)).astype(np.float32)
        recip_tile = np.ascontiguousarray(recip.reshape(NTILES, P).T)

        cf32_tile = np.concatenate(
            [
                rel.transpose(1, 0, 2).reshape(P, 2 * nslot).astype(np.float32),
                recip_tile.astype(np.float32),
            ],
            axis=1,
        )
        m = {
            "cf16": cf16_tile,
            "cf32": np.ascontiguousarray(cf32_tile),
            "identw": identw_tile,
        }
        s0 = 0
        for i, ch in enumerate(chunk_sizes):
            blk = srcg[s0 * P * 2 : (s0 + ch) * P * 2].reshape(ch, P, 2, FEAT)
            m[f"src{i}"] = np.ascontiguousarray(blk.transpose(1, 0, 2, 3))
            s0 += ch
        in_maps.append(m)

    return nc, in_maps


def _gather_output(results):
    blocks = []
    for c in range(NCORES):
        o = np.asarray(results[c]["out"], dtype=np.float32)  # [P, NTILES, 2, P]
        o = o.reshape(P, NTILES, FEAT).transpose(1, 0, 2).reshape(NTILES * P, FEAT)[
            :NPC
        ]
        blocks.append(o)
    return np.concatenate(blocks, axis=0)[:N_NODES]


def run(inputs, trace=False, **spmd_kwargs):
    from concourse.bass_utils import run_bass_kernel_spmd

    nc, in_maps = _prepare(inputs)
    res = run_bass_kernel_spmd(
        nc, in_maps, core_ids=list(range(NCORES)), trace=trace, **spmd_kwargs
    )
    return _gather_output(res.results), res


def kernel(**inputs) -> np.ndarray:
    out, _ = run(inputs, trace=False)
    return out


# revision 72
# speedup vs baseline: 1.0689x; 1.0689x over previous
"""GNN scatter-mean + Linear kernel for Trainium2, 8 NeuronCores.

Strategy (node-sharded, fp8 DoubleRow, no collectives):
  - CPU: sort edges by destination node, bucket per core (each core owns 1250
    contiguous nodes = 10 tiles of 128). Edge features are shipped RAW
    (unscaled) quantized to fp8 e4m3 with error-feedback (noise-shaped)
    rounding along each node's edge chain: the running quantization error is
    carried into the next edge of the same node, so the device-side segment
    sum sees only ~one ulp of error instead of sqrt(count) ulps. The 1/count
    mean division (and the fp8 range scale) is applied on-device per node
    after aggregation.
  - Slots hold 256 edges as [128 partitions, 2 packed] consumed by a single
    DoubleRow fp8 matmul (2 contractions of K=128 per instruction, 128 PE
    cycles per slot): identity slots (partition p, both halves -> node p) use
    one constant [128,2,128] fp8 weight tile loaded once (redundant Ldweights
    stripped post-compile); overflow edges use per-slot one-hot weights built
    on DVE (iota + is_equal, one op per packed half). PSUM accumulates fp32.
  - The whole edge stream (~10.7 MB/core) fits in SBUF, so every chunk DMA
    is triggered upfront with no pool recycling and the PE just follows the
    stream front. Consts are packed into 3 DMAs and the output into 2 that
    are emitted last, keeping the total DMA count low: past ~2 outstanding
    per DMAHW lane the tile scheduler injects cap-gate waits into engine
    streams that couple early compute to late chunk completions.
  - Per node tile (software-pipelined one tile behind the aggregation so the
    in-order PE queue never stalls on the finalize chain): evict PSUM on the
    ACT engine via activation-with-scale using the per-node 1/(count*SCALE)
    vector (fp16 out), transpose via PE, apply the 256x256 Linear (2 K-chunk
    fp16 matmuls; bias via a K=1 ones-row matmul only when nonzero), copy to
    the staging buffer on ACT, batched DMA out.
  - CPU: concatenate the 8 per-core [1250, 256] blocks.

At this point the kernel is bound by chip-aggregate HBM bandwidth (8 cores
x 10.7 MB streaming concurrently) plus ~13 us of fixed NEFF init/teardown;
the slowest core is set by DMA arbitration luck (measured ~60 us max,
vs 92.9 us for the fp16 identity-slot baseline).
"""

import sys

sys.path.insert(0, "/opt/trn_rl_repo")

from contextlib import ExitStack

import ml_dtypes
import numpy as np

N_NODES = 10000
N_EDGES = 320000
FEAT = 256
NCORES = 8
P = 128
NPC = (N_NODES + NCORES - 1) // NCORES  # 1250 nodes per core
NTILES = (NPC + P - 1) // P  # 10 node tiles per core
CH = 16  # slots per steady-state DMA chunk (16 * 128 * 2 * 256 * 1B = 1 MiB)
SCALE = 32.0  # fp8 range scale; folded into the on-device recip multiply
W_OVF = 1.05  # cost weight of an overflow slot vs an identity slot

FP8 = ml_dtypes.float8_e4m3fn


def _plan(dst):
    """Choose the shared program structure from the destination indices.

    Slots carry 256 edges ([128 partitions, 2 packed]). caps[t] identity
    slots cover up to 2*caps[t] edges per node; k_ovf[t] one-hot slots take
    the spill. Shared across all 8 cores so one SPMD program serves every
    core.
    """
    perm = np.argsort(dst, kind="stable")
    dst_sorted = dst[perm]
    counts = np.bincount(dst, minlength=N_NODES)

    tile_ranges = []
    for c in range(NCORES):
        rng = []
        for t in range(NTILES):
            n0 = c * NPC + t * P
            n1 = min(c * NPC + min((t + 1) * P, NPC), N_NODES)
            lo = int(np.searchsorted(dst_sorted, n0, side="left"))
            hi = int(np.searchsorted(dst_sorted, n1, side="left"))
            rng.append((lo, hi, n0, n1))
        tile_ranges.append(rng)

    caps, k_ovf = [], []
    for t in range(NTILES):
        cnts = [counts[rng[t][2] : rng[t][3]] for rng in tile_ranges]
        best = None
        for C in range(1, 129):
            ovf_slots = max(
                -(-int(np.maximum(cc - 2 * C, 0).sum()) // (2 * P)) if cc.size else 0
                for cc in cnts
            )
            cost = C + W_OVF * ovf_slots
            if best is None or cost < best[0]:
                best = (cost, C, ovf_slots)
        _, C, ovf_slots = best
        caps.append(C)
        k_ovf.append(ovf_slots)

    base = [0] * (NTILES + 1)
    cur = 0
    for t in range(NTILES):
        base[t] = cur
        cur += caps[t] + k_ovf[t]
    base[NTILES] = cur
    return perm, dst_sorted, counts, tile_ranges, caps, k_ovf, base, cur


def _chunk_schedule(nslot):
    """All chunks are triggered upfront. Progressive sizes keep the PE fed
    continuously from ~1.5us in (instead of waiting for a first big
    transfer), a small tail chunk keeps the last tile's wait short. Keep the
    chunk count modest: the tile scheduler round-robins every DMA over 8
    DMAHW lanes and inserts waits on the 2nd+ same-lane occupant, so deep
    chunk lists serialize on chunk completions (benign) but anything
    compute-gated (the out-writes) must come LAST in emission order."""
    head = [2, 2, 4, 8, 16]
    tail = [6, 4]
    sizes = []
    rem = nslot - sum(tail)
    for sz in head:
        if rem <= 0:
            break
        take = min(sz, rem)
        sizes.append(take)
        rem -= take
    n_steady = max(1, -(-rem // CH))
    for i in range(n_steady):
        take = rem // (n_steady - i)
        sizes.append(take)
        rem -= take
    for sz in tail:
        sizes.append(sz)
    assert sum(sizes) == nslot, sizes
    return sizes


def _slot_to_chunk(chunk_sizes):
    m = []
    for ci, sz in enumerate(chunk_sizes):
        for cl in range(sz):
            m.append((ci, cl))
    return m


def _quantize_ef(src, perm, dst_sorted, counts):
    """fp8 e4m3 quantization with per-(node,feature) error feedback.

    Edges are processed in sorted order; the rounding error of edge r of a
    node is added to edge r+1 before rounding, telescoping the segment-sum
    error down to the final edge's single rounding error. Vectorized across
    nodes by rank. Returns codes aligned with the SORTED edge order, plus a
    trailing all-zeros pad row (gather index N_EDGES)."""
    x = src[perm].astype(np.float32) * SCALE
    starts = np.searchsorted(dst_sorted, np.arange(N_NODES)).astype(np.int64)
    q = np.empty((N_EDGES + 1, FEAT), dtype=FP8)
    q[N_EDGES] = 0.0
    carry = np.zeros((N_NODES, FEAT), dtype=np.float32)
    maxc = int(counts.max())
    nodes_all = np.arange(N_NODES)
    for r in range(maxc):
        sel = nodes_all[counts > r]
        eidx = starts[sel] + r
        t = x[eidx] + carry[sel]
        np.clip(t, -239.0, 239.0, out=t)
        qv = t.astype(FP8)
        q[eidx] = qv
        carry[sel] = t - qv.astype(np.float32)
    return q


def _build_program(caps, k_ovf, base, chunk_sizes, nslot, has_bias, dedup=True):
    from concourse import bacc, mybir
    import concourse.tile as tile

    f32 = mybir.dt.float32
    f16 = mybir.dt.float16
    f8 = mybir.dt.float8e4
    eq = mybir.AluOpType.is_equal
    Ident = mybir.ActivationFunctionType.Identity
    DR = mybir.MatmulPerfMode.DoubleRow

    nc = bacc.Bacc("TRN2", target_bir_lowering=False, debug=False)

    src_drams = [
        nc.dram_tensor(f"src{i}", [P, ch, 2, FEAT], f8, kind="ExternalInput")
        for i, ch in enumerate(chunk_sizes)
    ]
    # consts are packed into 3 DMAs (f16 / f32 / fp8) to keep the total DMA
    # count low (DMAHW lane pressure, see _chunk_schedule).
    # cf16 segments of P: [iota | identt | wt(4)] + [bias(2) | ones-row]
    # (bias segments only when b is nonzero)
    nseg = 9 if has_bias else 6
    cf16_d = nc.dram_tensor("cf16", [P, nseg, P], f16, kind="ExternalInput")
    cf32_d = nc.dram_tensor("cf32", [P, 2 * nslot + NTILES], f32, kind="ExternalInput")
    identw_d = nc.dram_tensor("identw", [P, 2, P], f8, kind="ExternalInput")
    out_d = nc.dram_tensor("out", [P, NTILES, 2, P], f16, kind="ExternalOutput")

    with tile.TileContext(nc) as tc, ExitStack() as ctx:
        const = ctx.enter_context(tc.tile_pool(name="const", bufs=1))

        srcp = ctx.enter_context(
            tc.tile_pool(name="srcp", bufs=len(chunk_sizes))
        )
        ohp = ctx.enter_context(tc.tile_pool(name="ohp", bufs=12))
        meanp = ctx.enter_context(tc.tile_pool(name="meanp", bufs=2))
        mtp = ctx.enter_context(tc.tile_pool(name="mtp", bufs=2))
        outp = ctx.enter_context(tc.tile_pool(name="outp", bufs=1))
        ps_agg = ctx.enter_context(tc.tile_pool(name="ps_agg", bufs=2, space="PSUM"))
        ps_t = ctx.enter_context(tc.tile_pool(name="ps_t", bufs=2, space="PSUM"))
        ps_out = ctx.enter_context(tc.tile_pool(name="ps_out", bufs=2, space="PSUM"))

        identw_sb = const.tile([P, 2, P], f8)
        nc.scalar.dma_start(identw_sb[:], identw_d[:])
        cf16 = const.tile([P, nseg, P], f16)
        nc.scalar.dma_start(cf16[:], cf16_d[:])
        cf32 = const.tile([P, 2 * nslot + NTILES], f32)
        nc.scalar.dma_start(cf32[:], cf32_d[:])


        # the whole edge stream lives in SBUF: trigger every chunk upfront so
        # the DMA engines stream back-to-back with no flow-control coupling
        chunk_tiles = []
        for i, ch in enumerate(chunk_sizes):
            ct = srcp.tile([P, ch, 2, FEAT], f8, tag="src_chunk")
            nc.sync.dma_start(ct[:], src_drams[i][:])
            chunk_tiles.append(ct)

        s2c = _slot_to_chunk(chunk_sizes)

        # single output staging buffer; batched out-DMAs are emitted AFTER all
        # chunk triggers in scheduling order so the round-robin DMAHW lane
        # sems never make a chunk trigger wait on a compute-gated out-write
        ob_all = outp.tile([P, NTILES, 2, P], f16)
        OUT_SPLIT = 7

        def finalize(t, agg):
            # tile-finalize chain: PSUM-evict with the per-node 1/(count*S)
            # scale + the Linear. Evict/copy ops run on ACT (scalar) so the
            # DVE keeps its throughput for one-hot builds; bias is injected
            # as a K=1 matmul (ones-row x bias-row) that also initializes
            # the PSUM.
            mean = meanp.tile([P, FEAT], f16)
            rc = 2 * nslot + t
            nc.scalar.activation(mean[:], agg[:], Ident, scale=cf32[:, rc : rc + 1])
            tp = ps_t.tile([P, 2, P], f16)
            nc.tensor.transpose(tp[:, 0, :], mean[:, 0:P], cf16[:, 1, :])
            nc.tensor.transpose(tp[:, 1, :], mean[:, P : 2 * P], cf16[:, 1, :])
            mt = mtp.tile([P, 2, P], f16)
            nc.scalar.copy(mt[:], tp[:])
            op_ = ps_out.tile([P, 2, P], f32)
            if has_bias:
                nc.tensor.matmul(
                    op_[:], cf16[0:1, 8, :], cf16[0:1, 6:8, :], start=True, stop=False
                )
            nc.tensor.matmul(
                op_[:], mt[:, 0, :], cf16[:, 2:4, :], start=not has_bias, stop=False
            )
            nc.tensor.matmul(op_[:], mt[:, 1, :], cf16[:, 4:6, :], start=False, stop=True)
            nc.vector.tensor_copy(ob_all[:, t], op_[:])
            if t == OUT_SPLIT - 1:
                nc.scalar.dma_start(out_d[:, 0:OUT_SPLIT], ob_all[:, 0:OUT_SPLIT])
            elif t == NTILES - 1:
                nc.scalar.dma_start(
                    out_d[:, OUT_SPLIT:NTILES], ob_all[:, OUT_SPLIT:NTILES]
                )

        # software pipelining: tile t's finalize is emitted AFTER tile t+2's
        # aggregation matmuls, so the in-order PE queue never stalls waiting
        # for the ACT engine's eviction/copy of a previous tile
        from collections import deque

        pending = deque()
        for t in range(NTILES):
            agg = ps_agg.tile([P, FEAT], f32)
            kst = caps[t] + k_ovf[t]
            for k in range(kst):
                s = base[t] + k
                ci, cl = s2c[s]
                ct = chunk_tiles[ci]
                if k < caps[t]:
                    lhsT = identw_sb[:]
                else:
                    oh = ohp.tile([P, 2, P], f8)
                    nc.vector.tensor_scalar(
                        oh[:, 0, :], cf16[:, 0, :], cf32[:, 2 * s : 2 * s + 1], None, eq
                    )
                    nc.vector.tensor_scalar(
                        oh[:, 1, :], cf16[:, 0, :], cf32[:, 2 * s + 1 : 2 * s + 2], None, eq
                    )
                    lhsT = oh[:]
                nc.tensor.matmul(
                    agg[:],
                    lhsT,
                    ct[:, cl],
                    start=(k == 0),
                    stop=(k == kst - 1),
                    perf_mode=DR,
                )
            pending.append((t, agg))
            if len(pending) > 1:
                finalize(*pending.popleft())
        while pending:
            finalize(*pending.popleft())

    nc.compile()
    if dedup:
        _postprocess_module(nc)
    return nc


def _postprocess_module(nc):
    """Two post-compile rewrites of the module JSON:

    1. Remove back-to-back redundant Ldweights on the PE stream (same weights
       AP, no new semaphore obligations): identity-slot chains reload the
       same stationary operand; Matmult keeps the last loaded weights.
    2. Hoist the leading wait-free DMA triggers (first src chunks + consts)
       out of the Tile body into `main` ahead of the all-engine init barrier
       so the first bytes stream during engine init."""
    import orjson
    from concourse import mybir

    raw = nc.to_json()
    removed = 0
    for fn in raw["functions"]:
        for blk in fn["blocks"]:
            out = []
            last_sig = None
            enforced = {}  # sem id -> max wait value already enforced on PE
            for inst in blk["instructions"]:
                if inst.get("engine") == "PE":
                    sync = inst.get("sync_info") or {}
                    waits = sync.get("on_wait") or []
                    if inst.get("opcode") == "Ldweights":
                        ups = sync.get("on_update") or []
                        sig = orjson.dumps(
                            {
                                k: v
                                for k, v in inst.items()
                                if k not in ("name", "debug", "sync_info")
                            },
                            option=orjson.OPT_SORT_KEYS,
                        )
                        if (
                            sig == last_sig
                            and not ups
                            and all(
                                w.get("sync_type") == "semaphore"
                                and isinstance(w.get("wait_value"), int)
                                and enforced.get(w["id"], -1) >= w["wait_value"]
                                for w in waits
                            )
                        ):
                            removed += 1
                            continue
                        last_sig = sig
                    for w in waits:
                        if w.get("sync_type") == "semaphore" and isinstance(
                            w.get("wait_value"), int
                        ):
                            enforced[w["id"]] = max(
                                enforced.get(w["id"], -1), w["wait_value"]
                            )
                out.append(inst)
            blk["instructions"] = out

    for fn in raw["functions"]:
        blocks = {b["name"]: b for b in fn["blocks"]}
        main = blocks.get("main")
        body = None
        for b in fn["blocks"]:
            if b["name"] != "main" and len(b["instructions"]) > 100:
                body = b
        if main is None or body is None:
            continue
        hoist = []
        kept = []
        for idx, inst in enumerate(body["instructions"]):
            if len(hoist) >= 10 or idx > 60:
                kept.extend(body["instructions"][idx:])
                break
            sync = inst.get("sync_info") or {}
            if inst.get("opcode") == "DMACopy" and not (sync.get("on_wait") or []):
                hoist.append(inst)
            else:
                kept.append(inst)
        if not hoist:
            continue
        body["instructions"] = kept
        mi = main["instructions"]
        pos = next(
            (i for i, x in enumerate(mi) if x.get("opcode") == "Drain"), len(mi)
        )
        main["instructions"] = mi[:pos] + hoist + mi[pos:]

    nc.m = mybir.parse_bytes(orjson.dumps(raw))
    return removed


def _prepare(inputs, dedup=True):
    """CPU-side sharding: returns (nc, in_maps) ready for SPMD dispatch."""
    src = np.asarray(inputs["source_node_representation_with_coefficient"])
    edge_index = np.asarray(inputs["edge_index"])
    W = np.asarray(inputs["W"], dtype=np.float32)
    b = np.asarray(inputs["b"], dtype=np.float32)
    assert src.shape == (N_EDGES, FEAT) and edge_index.shape == (2, N_EDGES)

    dst = edge_index[1].astype(np.int64)
    perm, dst_sorted, counts, tile_ranges, caps, k_ovf, base, nslot = _plan(dst)

    q = _quantize_ef(src, perm, dst_sorted, counts)  # [E+1, F] fp8, sorted order

    chunk_sizes = _chunk_schedule(nslot)
    has_bias = bool(np.any(b != 0))
    nc = _build_program(
        caps, k_ovf, base, chunk_sizes, nslot, has_bias, dedup=dedup
    )

    # cf16 = [iota | identt | wt(4)] (+ [bias(2) | ones-row] when b != 0)
    nseg = 9 if has_bias else 6
    cf16_tile = np.zeros((P, nseg, P), dtype=np.float16)
    cf16_tile[:, 0, :] = np.arange(P, dtype=np.float16)[None, :]
    cf16_tile[:, 1, :] = np.eye(P, dtype=np.float16)
    cf16_tile[:, 2:6, :] = (
        W.T.reshape(2, P, 2, P).transpose(1, 0, 2, 3).reshape(P, 4, P)
    ).astype(np.float16)
    if has_bias:
        cf16_tile[:, 6:8, :] = np.broadcast_to(b, (P, FEAT)).reshape(P, 2, P).astype(
            np.float16
        )
        cf16_tile[0, 8, :] = 1.0
    identw_tile = np.zeros((P, 2, P), dtype=FP8)
    for j in range(2):
        identw_tile[np.arange(P), j, np.arange(P)] = 1.0

    # recip[p, t] = 1 / (max(count,1) * SCALE) for node t*128+p of this core
    pad = N_EDGES  # index of the all-zeros pad row in q

    in_maps = []
    for c in range(NCORES):
        pos = np.full((nslot, P, 2), pad, dtype=np.int64)  # sorted-order edge idx
        rel = np.zeros((nslot, P, 2), dtype=np.int64)
        for t in range(NTILES):
            lo, hi, n0, n1 = tile_ranges[c][t]
            n = hi - lo
            rows = n1 - n0
            b0 = base[t]
            C = caps[t]
            if n == 0:
                continue
            d_rel = dst_sorted[lo:hi] - n0  # sorted, in [0, rows)
            starts = np.searchsorted(d_rel, np.arange(rows))
            cnt_p = np.diff(np.append(starts, n))
            # identity slots: slot k half j, partition p <- edge 2k+j of node p
            kk = (2 * np.arange(C)[:, None, None] + np.arange(2)[None, None, :])
            valid = kk < cnt_p[None, :, None]  # [C, rows, 2]
            idx = np.minimum(starts[None, :, None] + kk, n - 1)
            pos[b0 : b0 + C, :rows] = np.where(valid, lo + idx, pad)
            # overflow edges: rank >= 2C within their node, packed densely
            rank = np.arange(n) - starts[d_rel]
            om = rank >= 2 * C
            novf = int(om.sum())
            if novf:
                ob0 = b0 + C
                tend = b0 + C + k_ovf[t]
                flat_pos = pos[ob0:tend].reshape(-1)
                flat_rel = rel[ob0:tend].reshape(-1)
                flat_pos[:novf] = lo + np.nonzero(om)[0]
                flat_rel[:novf] = d_rel[om]

        srcg = q[pos.reshape(-1)]  # [(nslot*P*2), F] fp8

        node0 = c * NPC
        cnt_core = np.zeros(NTILES * P, dtype=np.float64)
        ncv = min(NPC, N_NODES - node0)
        cnt_core[:ncv] = counts[node0 : node0 + ncv]
        recip = (1.0 / (np.maximum(cnt_core, 1.0) * SCALE)).astype(np.float32)
        recip_tile = np.ascontiguousarray(recip.reshape(NTILES, P).T)

        cf32_tile = np.concatenate(
            [
                rel.transpose(1, 0, 2).reshape(P, 2 * nslot).astype(np.float32),
                recip_tile.astype(np.float32),
            ],
            axis=1,
        )
        m = {
            "cf16": cf16_tile,
            "cf32": np.ascontiguousarray(cf32_tile),
            "identw": identw_tile,
        }
        s0 = 0
        for i, ch in enumerate(chunk_sizes):
            blk = srcg[s0 * P * 2 : (s0 + ch) * P * 2].reshape(ch, P, 2, FEAT)
            m[f"src{i}"] = np.ascontiguousarray(blk.transpose(1, 0, 2, 3))
            s0 += ch
        in_maps.append(m)

    return nc, in_maps


def _gather_output(results):
    blocks = []
    for c in range(NCORES):
        o = np.asarray(results[c]["out"], dtype=np.float32)  # [P, NTILES, 2, P]
        o = o.reshape(P, NTILES, FEAT).transpose(1, 0, 2).reshape(NTILES * P, FEAT)[
            :NPC
        ]
        blocks.append(o)
    return np.concatenate(blocks, axis=0)[:N_NODES]


def run(inputs, trace=False, **spmd_kwargs):
    from concourse.bass_utils import run_bass_kernel_spmd

    nc, in_maps = _prepare(inputs)
    res = run_bass_kernel_spmd(
        nc, in_maps, core_ids=list(range(NCORES)), trace=trace, **spmd_kwargs
    )
    return _gather_output(res.results), res


def kernel(**inputs) -> np.ndarray:
    out, _ = run(inputs, trace=False)
    return out


# revision 73
# speedup vs baseline: 1.1100x; 1.0385x over previous
"""GNN scatter-mean + Linear kernel for Trainium2, 8 NeuronCores.

Strategy (node-sharded, fp8 DoubleRow, no collectives):
  - CPU: sort edges by destination node, bucket per core (each core owns 1250
    contiguous nodes = 10 tiles of 128). Edge features are shipped RAW
    (unscaled) quantized to fp8 e4m3 with error-feedback (noise-shaped)
    rounding along each node's edge chain: the running quantization error is
    carried into the next edge of the same node, so the device-side segment
    sum sees only ~one ulp of error instead of sqrt(count) ulps. The 1/count
    mean division (and the fp8 range scale) is applied on-device per node
    after aggregation.
  - Slots hold 256 edges as [128 partitions, 2 packed] consumed by a single
    DoubleRow fp8 matmul (2 contractions of K=128 per instruction, 128 PE
    cycles per slot): identity slots (partition p, both halves -> node p) use
    one constant [128,2,128] fp8 weight tile loaded once (redundant Ldweights
    stripped post-compile); overflow edges use per-slot one-hot weights built
    on DVE (iota + is_equal, one op per packed half). PSUM accumulates fp32.
  - The whole edge stream (~10.7 MB/core) fits in SBUF, so every chunk DMA
    is triggered upfront with no pool recycling and the PE just follows the
    stream front. Consts are packed into 3 DMAs and the output into 2 that
    are emitted last, keeping the total DMA count low: past ~2 outstanding
    per DMAHW lane the tile scheduler injects cap-gate waits into engine
    streams that couple early compute to late chunk completions.
  - Per node tile (software-pipelined one tile behind the aggregation so the
    in-order PE queue never stalls on the finalize chain): evict PSUM on the
    ACT engine via activation-with-scale using the per-node 1/(count*SCALE)
    vector (fp16 out), transpose via PE, apply the 256x256 Linear (2 K-chunk
    fp16 matmuls; bias via a K=1 ones-row matmul only when nonzero), copy to
    the staging buffer on DVE, batched DMA out.
  - CPU: concatenate the 8 per-core [1250, 256] blocks.

At this point the kernel is bound by chip-aggregate HBM bandwidth (8 cores
x 10.7 MB streaming concurrently) plus ~13 us of fixed NEFF init/teardown;
the slowest core is set by DMA arbitration luck (measured ~60 us max,
vs 92.9 us for the fp16 identity-slot baseline).
"""

import sys

sys.path.insert(0, "/opt/trn_rl_repo")

from contextlib import ExitStack

import ml_dtypes
import numpy as np

N_NODES = 10000
N_EDGES = 320000
FEAT = 256
NCORES = 8
P = 128
NPC = (N_NODES + NCORES - 1) // NCORES  # 1250 nodes per core
NTILES = (NPC + P - 1) // P  # 10 node tiles per core
CH = 16  # slots per steady-state DMA chunk (16 * 128 * 2 * 256 * 1B = 1 MiB)
SCALE = 32.0  # fp8 range scale; folded into the on-device recip multiply
W_OVF = 1.05  # cost weight of an overflow slot vs an identity slot

FP8 = ml_dtypes.float8_e4m3fn


def _plan(dst):
    """Choose the shared program structure from the destination indices.

    Slots carry 256 edges ([128 partitions, 2 packed]). caps[t] identity
    slots cover up to 2*caps[t] edges per node; k_ovf[t] one-hot slots take
    the spill. Shared across all 8 cores so one SPMD program serves every
    core.
    """
    perm = np.argsort(dst, kind="stable")
    dst_sorted = dst[perm]
    counts = np.bincount(dst, minlength=N_NODES)

    tile_ranges = []
    for c in range(NCORES):
        rng = []
        for t in range(NTILES):
            n0 = c * NPC + t * P
            n1 = min(c * NPC + min((t + 1) * P, NPC), N_NODES)
            lo = int(np.searchsorted(dst_sorted, n0, side="left"))
            hi = int(np.searchsorted(dst_sorted, n1, side="left"))
            rng.append((lo, hi, n0, n1))
        tile_ranges.append(rng)

    caps, k_ovf = [], []
    for t in range(NTILES):
        cnts = [counts[rng[t][2] : rng[t][3]] for rng in tile_ranges]
        best = None
        for C in range(1, 129):
            ovf_slots = max(
                -(-int(np.maximum(cc - 2 * C, 0).sum()) // (2 * P)) if cc.size else 0
                for cc in cnts
            )
            cost = C + W_OVF * ovf_slots
            if best is None or cost < best[0]:
                best = (cost, C, ovf_slots)
        _, C, ovf_slots = best
        caps.append(C)
        k_ovf.append(ovf_slots)

    base = [0] * (NTILES + 1)
    cur = 0
    for t in range(NTILES):
        base[t] = cur
        cur += caps[t] + k_ovf[t]
    base[NTILES] = cur
    return perm, dst_sorted, counts, tile_ranges, caps, k_ovf, base, cur


def _chunk_schedule(nslot):
    """All chunks are triggered upfront. Progressive sizes keep the PE fed
    continuously from ~1.5us in (instead of waiting for a first big
    transfer), a small tail chunk keeps the last tile's wait short. Keep the
    chunk count modest: the tile scheduler round-robins every DMA over 8
    DMAHW lanes and inserts waits on the 2nd+ same-lane occupant, so deep
    chunk lists serialize on chunk completions (benign) but anything
    compute-gated (the out-writes) must come LAST in emission order."""
    head = [2, 2, 4, 8, 16]
    tail = [6, 4]
    sizes = []
    rem = nslot - sum(tail)
    for sz in head:
        if rem <= 0:
            break
        take = min(sz, rem)
        sizes.append(take)
        rem -= take
    n_steady = max(1, -(-rem // CH))
    for i in range(n_steady):
        take = rem // (n_steady - i)
        sizes.append(take)
        rem -= take
    for sz in tail:
        sizes.append(sz)
    assert sum(sizes) == nslot, sizes
    return sizes


def _slot_to_chunk(chunk_sizes):
    m = []
    for ci, sz in enumerate(chunk_sizes):
        for cl in range(sz):
            m.append((ci, cl))
    return m


def _quantize_ef(src, perm, dst_sorted, counts):
    """fp8 e4m3 quantization with per-(node,feature) error feedback.

    Edges are processed in sorted order; the rounding error of edge r of a
    node is added to edge r+1 before rounding, telescoping the segment-sum
    error down to the final edge's single rounding error. Vectorized across
    nodes by rank. Returns codes aligned with the SORTED edge order, plus a
    trailing all-zeros pad row (gather index N_EDGES)."""
    x = src[perm].astype(np.float32) * SCALE
    starts = np.searchsorted(dst_sorted, np.arange(N_NODES)).astype(np.int64)
    q = np.empty((N_EDGES + 1, FEAT), dtype=FP8)
    q[N_EDGES] = 0.0
    carry = np.zeros((N_NODES, FEAT), dtype=np.float32)
    maxc = int(counts.max())
    nodes_all = np.arange(N_NODES)
    for r in range(maxc):
        sel = nodes_all[counts > r]
        eidx = starts[sel] + r
        t = x[eidx] + carry[sel]
        np.clip(t, -239.0, 239.0, out=t)
        qv = t.astype(FP8)
        q[eidx] = qv
        carry[sel] = t - qv.astype(np.float32)
    return q


def _build_program(caps, k_ovf, base, chunk_sizes, nslot, has_bias, dedup=True):
    from concourse import bacc, mybir
    import concourse.tile as tile

    f32 = mybir.dt.float32
    f16 = mybir.dt.float16
    f8 = mybir.dt.float8e4
    eq = mybir.AluOpType.is_equal
    Ident = mybir.ActivationFunctionType.Identity
    DR = mybir.MatmulPerfMode.DoubleRow

    nc = bacc.Bacc("TRN2", target_bir_lowering=False, debug=False)

    src_drams = [
        nc.dram_tensor(f"src{i}", [P, ch, 2, FEAT], f8, kind="ExternalInput")
        for i, ch in enumerate(chunk_sizes)
    ]
    # consts are packed into 3 DMAs (f16 / f32 / fp8) to keep the total DMA
    # count low (DMAHW lane pressure, see _chunk_schedule).
    # cf16 segments of P: [iota | identt | wt(4)] + [bias(2) | ones-row]
    # (bias segments only when b is nonzero)
    nseg = 9 if has_bias else 6
    cf16_d = nc.dram_tensor("cf16", [P, nseg, P], f16, kind="ExternalInput")
    cf32_d = nc.dram_tensor("cf32", [P, 2 * nslot + NTILES], f32, kind="ExternalInput")
    identw_d = nc.dram_tensor("identw", [P, 2, P], f8, kind="ExternalInput")
    out_d = nc.dram_tensor("out", [P, NTILES, 2, P], f16, kind="ExternalOutput")

    with tile.TileContext(nc) as tc, ExitStack() as ctx:
        const = ctx.enter_context(tc.tile_pool(name="const", bufs=1))

        srcp = ctx.enter_context(
            tc.tile_pool(name="srcp", bufs=len(chunk_sizes))
        )
        ohp = ctx.enter_context(tc.tile_pool(name="ohp", bufs=12))
        meanp = ctx.enter_context(tc.tile_pool(name="meanp", bufs=2))
        mtp = ctx.enter_context(tc.tile_pool(name="mtp", bufs=2))
        outp = ctx.enter_context(tc.tile_pool(name="outp", bufs=1))
        ps_agg = ctx.enter_context(tc.tile_pool(name="ps_agg", bufs=2, space="PSUM"))
        ps_t = ctx.enter_context(tc.tile_pool(name="ps_t", bufs=2, space="PSUM"))
        ps_out = ctx.enter_context(tc.tile_pool(name="ps_out", bufs=2, space="PSUM"))

        identw_sb = const.tile([P, 2, P], f8)
        nc.scalar.dma_start(identw_sb[:], identw_d[:])
        cf16 = const.tile([P, nseg, P], f16)
        nc.scalar.dma_start(cf16[:], cf16_d[:])
        cf32 = const.tile([P, 2 * nslot + NTILES], f32)
        nc.scalar.dma_start(cf32[:], cf32_d[:])


        # the whole edge stream lives in SBUF: trigger every chunk upfront so
        # the DMA engines stream back-to-back with no flow-control coupling
        chunk_tiles = []
        for i, ch in enumerate(chunk_sizes):
            ct = srcp.tile([P, ch, 2, FEAT], f8, tag="src_chunk")
            nc.sync.dma_start(ct[:], src_drams[i][:])
            chunk_tiles.append(ct)

        s2c = _slot_to_chunk(chunk_sizes)

        # single output staging buffer; batched out-DMAs are emitted AFTER all
        # chunk triggers in scheduling order so the round-robin DMAHW lane
        # sems never make a chunk trigger wait on a compute-gated out-write
        ob_all = outp.tile([P, NTILES, 2, P], f16)
        OUT_SPLIT = 7

        def finalize(t, agg):
            # tile-finalize chain: PSUM-evict with the per-node 1/(count*S)
            # scale + the Linear. Evict/copy ops run on ACT (scalar) so the
            # DVE keeps its throughput for one-hot builds; bias is injected
            # as a K=1 matmul (ones-row x bias-row) that also initializes
            # the PSUM.
            mean = meanp.tile([P, FEAT], f16)
            rc = 2 * nslot + t
            nc.scalar.activation(mean[:], agg[:], Ident, scale=cf32[:, rc : rc + 1])
            tp = ps_t.tile([P, 2, P], f16)
            nc.tensor.transpose(tp[:, 0, :], mean[:, 0:P], cf16[:, 1, :])
            nc.tensor.transpose(tp[:, 1, :], mean[:, P : 2 * P], cf16[:, 1, :])
            mt = mtp.tile([P, 2, P], f16)
            nc.scalar.copy(mt[:], tp[:])
            op_ = ps_out.tile([P, 2, P], f32)
            if has_bias:
                nc.tensor.matmul(
                    op_[:], cf16[0:1, 8, :], cf16[0:1, 6:8, :], start=True, stop=False
                )
            nc.tensor.matmul(
                op_[:], mt[:, 0, :], cf16[:, 2:4, :], start=not has_bias, stop=False
            )
            nc.tensor.matmul(op_[:], mt[:, 1, :], cf16[:, 4:6, :], start=False, stop=True)
            nc.vector.tensor_copy(ob_all[:, t], op_[:])
            if t == OUT_SPLIT - 1:
                nc.scalar.dma_start(out_d[:, 0:OUT_SPLIT], ob_all[:, 0:OUT_SPLIT])
            elif t == NTILES - 1:
                nc.scalar.dma_start(
                    out_d[:, OUT_SPLIT:NTILES], ob_all[:, OUT_SPLIT:NTILES]
                )

        # software pipelining: tile t's finalize is emitted AFTER tile t+2's
        # aggregation matmuls, so the in-order PE queue never stalls waiting
        # for the ACT engine's eviction/copy of a previous tile
        from collections import deque

        pending = deque()
        for t in range(NTILES):
            agg = ps_agg.tile([P, FEAT], f32)
            kst = caps[t] + k_ovf[t]
            for k in range(kst):
                s = base[t] + k
                ci, cl = s2c[s]
                ct = chunk_tiles[ci]
                if k < caps[t]:
                    lhsT = identw_sb[:]
                else:
                    oh = ohp.tile([P, 2, P], f8)
                    nc.vector.tensor_scalar(
                        oh[:, 0, :], cf16[:, 0, :], cf32[:, 2 * s : 2 * s + 1], None, eq
                    )
                    nc.vector.tensor_scalar(
                        oh[:, 1, :], cf16[:, 0, :], cf32[:, 2 * s + 1 : 2 * s + 2], None, eq
                    )
                    lhsT = oh[:]
                nc.tensor.matmul(
                    agg[:],
                    lhsT,
                    ct[:, cl],
                    start=(k == 0),
                    stop=(k == kst - 1),
                    perf_mode=DR,
                )
            pending.append((t, agg))
            if len(pending) > 1:
                finalize(*pending.popleft())
        while pending:
            finalize(*pending.popleft())

    nc.compile()
    if dedup:
        _postprocess_module(nc)
    return nc


def _postprocess_module(nc):
    """Two post-compile rewrites of the module JSON:

    1. Remove back-to-back redundant Ldweights on the PE stream (same weights
       AP, no new semaphore obligations): identity-slot chains reload the
       same stationary operand; Matmult keeps the last loaded weights.
    2. Hoist the leading wait-free DMA triggers (first src chunks + consts)
       out of the Tile body into `main` ahead of the all-engine init barrier
       so the first bytes stream during engine init."""
    import orjson
    from concourse import mybir

    raw = nc.to_json()
    removed = 0
    for fn in raw["functions"]:
        for blk in fn["blocks"]:
            out = []
            last_sig = None
            enforced = {}  # sem id -> max wait value already enforced on PE
            for inst in blk["instructions"]:
                if inst.get("engine") == "PE":
                    sync = inst.get("sync_info") or {}
                    waits = sync.get("on_wait") or []
                    if inst.get("opcode") == "Ldweights":
                        ups = sync.get("on_update") or []
                        sig = orjson.dumps(
                            {
                                k: v
                                for k, v in inst.items()
                                if k not in ("name", "debug", "sync_info")
                            },
                            option=orjson.OPT_SORT_KEYS,
                        )
                        if (
                            sig == last_sig
                            and not ups
                            and all(
                                w.get("sync_type") == "semaphore"
                                and isinstance(w.get("wait_value"), int)
                                and enforced.get(w["id"], -1) >= w["wait_value"]
                                for w in waits
                            )
                        ):
                            removed += 1
                            continue
                        last_sig = sig
                    for w in waits:
                        if w.get("sync_type") == "semaphore" and isinstance(
                            w.get("wait_value"), int
                        ):
                            enforced[w["id"]] = max(
                                enforced.get(w["id"], -1), w["wait_value"]
                            )
                out.append(inst)
            blk["instructions"] = out

    for fn in raw["functions"]:
        blocks = {b["name"]: b for b in fn["blocks"]}
        main = blocks.get("main")
        body = None
        for b in fn["blocks"]:
            if b["name"] != "main" and len(b["instructions"]) > 100:
                body = b
        if main is None or body is None:
            continue
        hoist = []
        kept = []
        for idx, inst in enumerate(body["instructions"]):
            if len(hoist) >= 10 or idx > 60:
                kept.extend(body["instructions"][idx:])
                break
            sync = inst.get("sync_info") or {}
            if inst.get("opcode") == "DMACopy" and not (sync.get("on_wait") or []):
                hoist.append(inst)
            else:
                kept.append(inst)
        if not hoist:
            continue
        body["instructions"] = kept
        mi = main["instructions"]
        pos = next(
            (i for i, x in enumerate(mi) if x.get("opcode") == "Drain"), len(mi)
        )
        main["instructions"] = mi[:pos] + hoist + mi[pos:]

    nc.m = mybir.parse_bytes(orjson.dumps(raw))
    return removed


def _prepare(inputs, dedup=True):
    """CPU-side sharding: returns (nc, in_maps) ready for SPMD dispatch."""
    src = np.asarray(inputs["source_node_representation_with_coefficient"])
    edge_index = np.asarray(inputs["edge_index"])
    W = np.asarray(inputs["W"], dtype=np.float32)
    b = np.asarray(inputs["b"], dtype=np.float32)
    assert src.shape == (N_EDGES, FEAT) and edge_index.shape == (2, N_EDGES)

    dst = edge_index[1].astype(np.int64)
    perm, dst_sorted, counts, tile_ranges, caps, k_ovf, base, nslot = _plan(dst)

    q = _quantize_ef(src, perm, dst_sorted, counts)  # [E+1, F] fp8, sorted order

    chunk_sizes = _chunk_schedule(nslot)
    has_bias = bool(np.any(b != 0))
    nc = _build_program(
        caps, k_ovf, base, chunk_sizes, nslot, has_bias, dedup=dedup
    )

    # cf16 = [iota | identt | wt(4)] (+ [bias(2) | ones-row] when b != 0)
    nseg = 9 if has_bias else 6
    cf16_tile = np.zeros((P, nseg, P), dtype=np.float16)
    cf16_tile[:, 0, :] = np.arange(P, dtype=np.float16)[None, :]
    cf16_tile[:, 1, :] = np.eye(P, dtype=np.float16)
    cf16_tile[:, 2:6, :] = (
        W.T.reshape(2, P, 2, P).transpose(1, 0, 2, 3).reshape(P, 4, P)
    ).astype(np.float16)
    if has_bias:
        cf16_tile[:, 6:8, :] = np.broadcast_to(b, (P, FEAT)).reshape(P, 2, P).astype(
            np.float16
        )
        cf16_tile[0, 8, :] = 1.0
    identw_tile = np.zeros((P, 2, P), dtype=FP8)
    for j in range(2):
        identw_tile[np.arange(P), j, np.arange(P)] = 1.0

    # recip[p, t] = 1 / (max(count,1) * SCALE) for node t*128+p of this core
    pad = N_EDGES  # index of the all-zeros pad row in q

    in_maps = []
    for c in range(NCORES):
        pos = np.full((nslot, P, 2), pad, dtype=np.int64)  # sorted-order edge idx
        rel = np.zeros((nslot, P, 2), dtype=np.int64)
        for t in range(NTILES):
            lo, hi, n0, n1 = tile_ranges[c][t]
            n = hi - lo
            rows = n1 - n0
            b0 = base[t]
            C = caps[t]
            if n == 0:
                continue
            d_rel = dst_sorted[lo:hi] - n0  # sorted, in [0, rows)
            starts = np.searchsorted(d_rel, np.arange(rows))
            cnt_p = np.diff(np.append(starts, n))
            # identity slots: slot k half j, partition p <- edge 2k+j of node p
            kk = (2 * np.arange(C)[:, None, None] + np.arange(2)[None, None, :])
            valid = kk < cnt_p[None, :, None]  # [C, rows, 2]
            idx = np.minimum(starts[None, :, None] + kk, n - 1)
            pos[b0 : b0 + C, :rows] = np.where(valid, lo + idx, pad)
            # overflow edges: rank >= 2C within their node, packed densely
            rank = np.arange(n) - starts[d_rel]
            om = rank >= 2 * C
            novf = int(om.sum())
            if novf:
                ob0 = b0 + C
                tend = b0 + C + k_ovf[t]
                flat_pos = pos[ob0:tend].reshape(-1)
                flat_rel = rel[ob0:tend].reshape(-1)
                flat_pos[:novf] = lo + np.nonzero(om)[0]
                flat_rel[:novf] = d_rel[om]

        srcg = q[pos.reshape(-1)]  # [(nslot*P*2), F] fp8

        node0 = c * NPC
        cnt_core = np.zeros(NTILES * P, dtype=np.float64)
        ncv = min(NPC, N_NODES - node0)
        cnt_core[:ncv] = counts[node0 : node0 + ncv]
        recip = (1.0 / (np.maximum(cnt_core, 1.0) * SCALE)).astype(np.float32)
        recip_tile = np.ascontiguousarray(recip.reshape(NTILES, P).T)

        cf32_tile = np.concatenate(
            [
                rel.transpose(1, 0, 2).reshape(P, 2 * nslot).astype(np.float32),
                recip_tile.astype(np.float32),
            ],
            axis=1,
        )
        m = {
            "cf16": cf16_tile,
            "cf32": np.ascontiguousarray(cf32_tile),
            "identw": identw_tile,
        }
        s0 = 0
        for i, ch in enumerate(chunk_sizes):
            blk = srcg[s0 * P * 2 : (s0 + ch) * P * 2].reshape(ch, P, 2, FEAT)
            m[f"src{i}"] = np.ascontiguousarray(blk.transpose(1, 0, 2, 3))
            s0 += ch
        in_maps.append(m)

    return nc, in_maps


def _gather_output(results):
    blocks = []
    for c in range(NCORES):
        o = np.asarray(results[c]["out"], dtype=np.float32)  # [P, NTILES, 2, P]
        o = o.reshape(P, NTILES, FEAT).transpose(1, 0, 2).reshape(NTILES * P, FEAT)[
            :NPC
        ]
        blocks.append(o)
    return np.concatenate(blocks, axis=0)[:N_NODES]


def run(inputs, trace=False, **spmd_kwargs):
    from concourse.bass_utils import run_bass_kernel_spmd

    nc, in_maps = _prepare(inputs)
    res = run_bass_kernel_spmd(
        nc, in_maps, core_ids=list(range(NCORES)), trace=trace, **spmd_kwargs
    )
    return _gather_output(res.results), res


def kernel(**inputs) -> np.ndarray:
    out, _ = run(inputs, trace=False)
    return out
